# revision 39
# baseline (speedup 1.0000x reference)
"""Trainium2 Bass kernel for multi-head attention (B=2, S=2048, D=1024, H=16, HD=64).

Sharding: hybrid DP2 x TP4. Cores 0-3 own batch 0, cores 4-7 batch 1; within a
batch each core owns 4 heads as two head-PAIRS (hp=0,1), each pair occupying
the two 64-partition halves of the attention pipeline.

Everything is fp16 (PE rate identical to bf16, better mantissa). Per core:
  - q^T,k^T,v^T projections [hd-pair=128, seq=2048] per head-pair
  - v^T -> [1|pad63|vA|1|pad63|vB] blocks via PE transpose; the ones/pad
    columns make the attended matmul emit softmax denominators on PSUM
    partitions 0:64 for free
  - scores^T = K @ Q^T in [key, query] layout (K=64 matmuls; NOTE: PE
    row-group concurrency does NOT engage when K=128 matmuls are interleaved
    in the stream, and accumulation groups spanning different row-group
    configs crash the device - so scores run serial full-rate)
  - exp is split per group across BOTH engines: head A on ScalarE activation
    (exact, scale=1/8 fused, feeds the attended pipeline soonest), head B on
    DVE via a Schraudolph integer exp: i16 = round(s*184.665 + 15315.5)
    bit-cast as fp16 ~= exp(s/8) - one tensor_scalar op reading PSUM
    directly. The ~1.5% per-weight noise on half the heads costs ~0.7% rel
    err (measured total 0.0086 vs 0.02 budget).
  - attended^T accumulated over 16 key tiles with 3-4 group slack behind exp;
    normalization via reciprocal_approx_fast on the denominator rows + one
    tensor_tensor mult reading PSUM (operands at different base partitions
    are legal)
  - output projection accumulates BOTH head pairs (K=128 x2) into PSUM,
    drained by ScalarE Copy / DVE, DMA'd out as fp16 partials
Host sums the 4 partial outputs per batch and adds the bias.

The PE stream is hand-woven (static per-engine schedule): projections ride in
the idle slices of the attention group loop, head-B attended + normalize are
software-pipelined one iteration later and the output projection two behind
(one s-tile per late group), ~30 warmup matmuls at t=0 spin the PE out of its
low p-state while the first DMAs land (x chunk0 issued from the GpSimd queue,
25ns/issue vs 565 on sync), and a dummy activation preloads the exp table.
"""

import numpy as np

import concourse.bacc as bacc
import concourse.tile as tile
import concourse.mybir as mybir
from concourse.bass_utils import run_bass_kernel_spmd
from concourse.masks import make_identity

B, S, D = 2, 2048, 1024
H, HD = 16, 64
FEA = H * HD  # 1024
NCORES = 8

DT_TILES = 8      # 1024 contraction dim / 128
JT = 16           # key tiles of 128
IB = 4            # query blocks of 512
VW = 256          # v storage width per s-tile: [1|pad63|vA(64) | 1|pad63|vB(64)]

F16 = mybir.dt.float16
F32 = mybir.dt.float32
I16 = mybir.dt.int16
AF = mybir.ActivationFunctionType
ALU = mybir.AluOpType

# Schraudolph exp-as-int16-bit-pattern constants for fp16 output:
# i16 = round(score * (2^10 * log2(e) / 8) + (15 * 2^10 - 44.5))
EXP_A = 184.66495
EXP_B = 15315.5

_NC_CACHE = {}


def _emit(tc, xT, wqk, wv, wo0, wo1, out):
    nc = tc.nc
    with (
        tc.tile_pool(name="consts", bufs=1) as consts,
        tc.tile_pool(name="stp", bufs=4) as stp,
        tc.tile_pool(name="small", bufs=6) as small,
        tc.tile_pool(name="outsb", bufs=6) as outsb,
        tc.tile_pool(name="ps_sc", bufs=2, space="PSUM") as ps_sc,
        tc.tile_pool(name="ps_att", bufs=2, space="PSUM") as ps_att,
        tc.tile_pool(name="ps_misc", bufs=2, space="PSUM") as ps_misc,
    ):
        xts = [consts.tile([128, S], F16, name=f"xt{i}", tag=f"xt{i}") for i in range(DT_TILES)]
        wqks = [consts.tile([128, 512], F16, name=f"wqk{i}", tag=f"wqk{i}") for i in range(DT_TILES)]
        wvs = [consts.tile([128, 256], F16, name=f"wv{i}", tag=f"wv{i}") for i in range(DT_TILES)]
        wos = [consts.tile([128, D], F16, name=f"wo{p}", tag=f"wo{p}") for p in range(2)]
        qTs = [consts.tile([128, S], F16, name=f"qT{p}", tag=f"qT{p}") for p in range(2)]
        kTs = [consts.tile([128, S], F16, name=f"kT{p}", tag=f"kT{p}") for p in range(2)]
        vTs = [consts.tile([128, S], F16, name=f"vT{p}", tag=f"vT{p}") for p in range(2)]
        vsbs = [consts.tile([128, JT * VW], F16, name=f"v{p}", tag=f"v{p}") for p in range(2)]
        attTs = [consts.tile([128, S], F16, name=f"attT{p}", tag=f"attT{p}") for p in range(2)]
        ident = consts.tile([128, 128], F16, tag="ident")

        junk = consts.tile([128, 128], F16, tag="junk")
        nc.vector.memset(junk, 0.001)
        make_identity(nc, ident)

        # PE warmup: dependency-free matmuls spin the tensor engine out of
        # its cold p-state while the first xT/weight DMAs are in flight.
        wm = ps_misc.tile([128, 128], F32, name="warm", tag="misc")
        for _ in range(30):
            nc.tensor.matmul(wm, lhsT=junk, rhs=junk, start=True, stop=True)
        # Dummy activation forces the exp table load (~2.7us) to overlap the
        # input DMAs instead of stalling the first real exp.
        da = outsb.tile([128, 64], F16, name="dummy_act", tag="osb")
        nc.scalar.activation(out=da, in_=junk[:, 0:64], func=AF.Exp, scale=1.0)

        # Input DMAs: first 512-col chunk of x^T + qk weights gate the first
        # projection; the rest rides behind.
        for dt in range(DT_TILES):
            nc.gpsimd.dma_start(out=xts[dt][:, 0:512], in_=xT[dt * 128:(dt + 1) * 128, 0:512])
            nc.sync.dma_start(out=wqks[dt], in_=wqk[dt * 128:(dt + 1) * 128, :])
        for dt in range(DT_TILES):
            nc.sync.dma_start(out=wvs[dt], in_=wv[dt * 128:(dt + 1) * 128, :])
            nc.sync.dma_start(out=xts[dt][:, 512:1024], in_=xT[dt * 128:(dt + 1) * 128, 512:1024])
        for dt in range(DT_TILES):
            nc.sync.dma_start(out=xts[dt][:, 1024:S], in_=xT[dt * 128:(dt + 1) * 128, 1024:S])
        nc.sync.dma_start(out=wos[0], in_=wo0[:, :])
        nc.sync.dma_start(out=wos[1], in_=wo1[:, :])
        for p in range(2):
            nc.gpsimd.memset(vsbs[p], 1.0)  # presets the ones/pad columns

        def emit_proj_nb(hp, half, nb, dst):
            # one 512-col block of a q^T (half=0) / k^T (half=1) projection
            scol = nb * 512
            base = half * 256 + hp * 128
            ps = ps_misc.tile([128, 512], F32, name="psp", tag="misc")
            for dt in range(DT_TILES):
                nc.tensor.matmul(
                    ps,
                    lhsT=wqks[dt][:, base:base + 128],
                    rhs=xts[dt][:, scol:scol + 512],
                    start=(dt == 0),
                    stop=(dt == DT_TILES - 1),
                )
            nc.scalar.activation(out=dst[:, scol:scol + 512], in_=ps, func=AF.Copy)

        def emit_q_nb(hp, nb):
            emit_proj_nb(hp, 0, nb, qTs[hp])

        def emit_k_nb(hp, nb):
            emit_proj_nb(hp, 1, nb, kTs[hp])

        def emit_vT_nb(hp, nb):
            scol = nb * 512
            ps = ps_misc.tile([128, 512], F32, name="psp", tag="misc")
            for dt in range(DT_TILES):
                nc.tensor.matmul(
                    ps,
                    lhsT=wvs[dt][:, hp * 128:(hp + 1) * 128],
                    rhs=xts[dt][:, scol:scol + 512],
                    start=(dt == 0),
                    stop=(dt == DT_TILES - 1),
                )
            nc.scalar.activation(out=vTs[hp][:, scol:scol + 512], in_=ps, func=AF.Copy)

        def emit_v2(hp, j):
            # PE-transpose s-tiles (2j, 2j+1) of v^T into natural [s, f] order,
            # then one strided DVE copy lands both as [1|pad|vA | 1|pad|vB].
            ps = ps_misc.tile([128, 256], F16, name="psv", tag="misc")
            for k2 in range(2):
                st = 2 * j + k2
                nc.tensor.transpose(
                    ps[:, k2 * 128:(k2 + 1) * 128],
                    vTs[hp][:, st * 128:(st + 1) * 128],
                    ident,
                )
            src = ps.rearrange("p (st two c) -> p st two c", st=2, two=2)
            dst = vsbs[hp][:, j * 512:(j + 1) * 512].rearrange(
                "p (st two c) -> p st two c", st=2, two=2
            )[:, :, :, 64:128]
            nc.vector.tensor_copy(out=dst, in_=src)

        def emit_outproj_ib(ib):
            # out rows [ib*512, +512): accumulate BOTH head pairs (K=128 each)
            for st in range(ib * 4, ib * 4 + 4):
                emit_outproj_st(st, split=True)

        def emit_outproj_st(st, split=False):
            if True:
                osb = outsb.tile([128, 1024], F16, name="osb", tag="osb")
                for db in range(2):
                    ps = ps_misc.tile([128, 512], F32, name="pso", tag="misc")
                    for hp2 in range(2):
                        nc.tensor.matmul(
                            ps,
                            lhsT=attTs[hp2][:, st * 128:(st + 1) * 128],
                            rhs=wos[hp2][:, db * 512:(db + 1) * 512],
                            start=(hp2 == 0),
                            stop=(hp2 == 1),
                        )
                    if split and db == 1:
                        nc.vector.tensor_copy(
                            out=osb[:, db * 512:(db + 1) * 512], in_=ps
                        )
                    else:
                        nc.scalar.activation(
                            out=osb[:, db * 512:(db + 1) * 512], in_=ps, func=AF.Copy
                        )
                nc.sync.dma_start(
                    out=out[st * 128:(st + 1) * 128, :], in_=osb
                )

        def normalize(hp, ib, h, att_ps):
            icol = ib * 512
            # PSUM partitions 0:64 all hold the softmax denominator (ones AND
            # pad columns of the v tile are 1.0); the tensor_tensor mult
            # tolerates operands at different base partitions.
            rb = small.tile([64, 512], F32, name="rb", tag="rb")
            nc.vector.reciprocal_approx_fast(out=rb, in_=att_ps[0:64, :])
            dst = (attTs[hp][0:64, icol:icol + 512] if h == 0
                   else attTs[hp][64:128, icol:icol + 512])
            nc.vector.tensor_tensor(
                out=dst,
                in0=att_ps[64:128, :],
                in1=rb,
                op=ALU.mult,
            )

        # (iteration, slot) -> prerequisite projection blocks to emit there.
        WEAVE = {
            (0, 1): (("vT", (0, 1)),),
            (0, 2): (("k", (0, 1)),),
            (0, 3): (("q", (0, 1)), ("vT", (0, 2))),
            (0, 4): (("k", (0, 2)),),
            (0, 5): (("q", (0, 2)), ("vT", (0, 3))),
            (0, 6): (("k", (0, 3)),),
            (0, 7): (("q", (0, 3)),),
            (1, 0): (("k", (1, 0)),),
            (1, 2): (("k", (1, 1)),),
            (1, 4): (("k", (1, 2)),),
            (1, 6): (("k", (1, 3)),),
            (1, 7): (("q", (1, 0)),),
            (2, 0): (("vT", (1, 0)),),
            (2, 2): (("vT", (1, 1)),),
            (2, 4): (("vT", (1, 2)),),
            (2, 6): (("vT", (1, 3)),),
            (3, 0): (("v2", (1, 0)),),
            (3, 1): (("v2", (1, 1)),),
            (3, 2): (("v2", (1, 2)),),
            (3, 3): (("v2", (1, 3)),),
            (3, 4): (("v2", (1, 4)),),
            (3, 5): (("v2", (1, 5)),),
            (4, 0): (("q", (1, 1)), ("v2", (1, 6))),
            (4, 2): (("v2", (1, 7)),),
            (5, 0): (("q", (1, 2)),),
            (6, 0): (("q", (1, 3)),),
        }

        # Head B's attended matmuls + normalize + output projection of
        # iteration i are software-pipelined into the START of iteration i+1.
        pend = {}
        oproj_due = []

        # attB(prev) runs in stages 1-3 of the next iteration (stage 0 left
        # free so the previous iteration's attA PSUM drain can release a slot)
        PEND_JTS = {1: (0, 8), 2: (8, 12), 3: (12, JT)}

        def weave_pending(stage):
            if "hp" not in pend or stage == 0:
                return
            php, pib, pstB = pend["hp"], pend["ib"], pend["stB"]
            if stage == 1:
                attB_ps = ps_att.tile([128, 512], F32, name="attps", tag="att")
                pend["ps"] = attB_ps
            else:
                attB_ps = pend["ps"]
            lo, hi = PEND_JTS[stage]
            for jt in range(lo, hi):
                nc.tensor.matmul(
                    attB_ps[0:128, :],
                    lhsT=vsbs[php][:, jt * VW + 128: jt * VW + 256],
                    rhs=pstB[:, jt * 512:(jt + 1) * 512],
                    start=(jt == 0),
                    stop=(jt == JT - 1),
                )
            if stage == 3:
                normalize(php, pib, 1, pend["ps"])
                if php == 1:
                    oproj_due.append(pib)
                pend.clear()

        def emit_exp(i, g, sc, st_exp, g0, gw, dve):
            if dve:
                o = st_exp[:, g0 * 512:(g0 + gw) * 512].bitcast(I16)
                nc.vector.tensor_scalar(
                    out=o,
                    in0=sc[:, 0:gw * 512],
                    scalar1=EXP_A,
                    scalar2=EXP_B,
                    op0=ALU.mult,
                    op1=ALU.add,
                )
            else:
                nc.scalar.activation(
                    out=st_exp[:, g0 * 512:(g0 + gw) * 512],
                    in_=sc[:, 0:gw * 512],
                    func=AF.Exp,
                    scale=0.125,
                )

        def emit_att_jts(hp, ps, stX, half, jts):
            # attended accumulation matmuls; half=0 -> head A v block, 1 -> B
            for jt in jts:
                nc.tensor.matmul(
                    ps[0:128, :],
                    lhsT=vsbs[hp][:, jt * VW + half * 128: jt * VW + half * 128 + 128],
                    rhs=stX[:, jt * 512:(jt + 1) * 512],
                    start=(jt == 0),
                    stop=(jt == JT - 1),
                )

        def emit_attention_ib(hp, ib):
            i = hp * 4 + ib
            last = i == 7
            icol = ib * 512
            stA = stp.tile([128, JT * 512], F16, name="stA", tag="st")
            stB = stp.tile([128, JT * 512], F16, name="stB", tag="st")
            attA_ps = None
            attB_ps = None
            for g0 in range(0, JT, 2):
                g = g0 // 2
                gw = 2
                if g < 4:
                    weave_pending(g)
                for kind, arg in WEAVE.get((i, g), ()):
                    if kind == "q":
                        emit_q_nb(*arg)
                    elif kind == "k":
                        emit_k_nb(*arg)
                    elif kind == "vT":
                        emit_vT_nb(*arg)
                    elif kind == "v2":
                        emit_v2(*arg)
                if attA_ps is None:
                    attA_ps = ps_att.tile([128, 512], F32, name="attps", tag="att")
                # attended head A consumes exps three groups back (slack so
                # the PE never waits on the exp engines); group 7 catches up
                # an extra pair so normalize fires sooner after the loop
                if g0 >= 6:
                    emit_att_jts(hp, attA_ps, stA, 0, (g0 - 6, g0 - 5))
                if g == 7:
                    emit_att_jts(hp, attA_ps, stA, 0, (10, 11))
                # output projection of the iteration-before-last, one s-tile
                # per group (spread so it never waits on fresh attT writes)
                if g >= 4 and oproj_due:
                    emit_outproj_st(oproj_due[0] * 4 + (g - 4))
                    if g == 7:
                        oproj_due.pop(0)
                scA = ps_sc.tile([128, 1024], F32, name="scA", tag="sc")
                scB = ps_sc.tile([128, 1024], F32, name="scB", tag="sc")
                for idx in range(gw):
                    jt = g0 + idx
                    # head A split into two chained K=32 row groups, head B
                    # one K=64 group (base partition 96 is not encodable):
                    # 3 concurrent PE row-group streams, each one's
                    # LDWEIGHTS hides under the others' streaming
                    for hsl, sc, st_, sp_ in (
                        (slice(0, 64), scA, True, True),
                        (slice(64, 128), scB, True, True),
                    ):
                        nc.tensor.matmul(
                            sc[:, idx * 512:(idx + 1) * 512],
                            lhsT=kTs[hp][hsl, jt * 128:(jt + 1) * 128],
                            rhs=qTs[hp][hsl, icol:icol + 512],
                            start=st_,
                            stop=sp_,
                        )
                emit_exp(i, g, scA, stA, g0, gw, False)
                emit_exp(i, g, scB, stB, g0, gw, True)
                # hp0/it0's v tiles just in time for the attended matmuls
                if i == 0:
                    emit_v2(0, g)
                if last and g >= 4:
                    # final iteration: head B attended inline (no next
                    # iteration to pipeline into); backlog burst at g4
                    if attB_ps is None:
                        attB_ps = ps_att.tile([128, 512], F32, name="attps", tag="att")
                        emit_att_jts(hp, attB_ps, stB, 1, range(0, 4))
                    emit_att_jts(hp, attB_ps, stB, 1, (g0 - 4, g0 - 3))

            emit_att_jts(hp, attA_ps, stA, 0, range(JT - 4, JT))
            normalize(hp, ib, 0, attA_ps)

            if last:
                emit_att_jts(hp, attB_ps, stB, 1, range(JT - 4, JT))
                normalize(hp, ib, 1, attB_ps)
                emit_outproj_ib(ib)
            else:
                pend.update(hp=hp, ib=ib, stB=stB)

        # prologue projections (gated only on the first x^T chunk)
        emit_k_nb(0, 0)
        emit_q_nb(0, 0)
        emit_vT_nb(0, 0)

        for hp in range(2):
            for ib in range(IB):
                emit_attention_ib(hp, ib)
        for _stage in range(4):
            weave_pending(_stage)


def build_nc():
    if "nc" in _NC_CACHE:
        return _NC_CACHE["nc"]
    nc = bacc.Bacc("TRN2", debug=False, num_devices=NCORES)
    xT = nc.dram_tensor("xT", [D, S], F16, kind="ExternalInput").ap()
    wqk = nc.dram_tensor("wqk", [D, 512], F16, kind="ExternalInput").ap()
    wv = nc.dram_tensor("wv", [D, 256], F16, kind="ExternalInput").ap()
    wo0 = nc.dram_tensor("wo0", [128, D], F16, kind="ExternalInput").ap()
    wo1 = nc.dram_tensor("wo1", [128, D], F16, kind="ExternalInput").ap()
    out = nc.dram_tensor("out", [S, D], F16, kind="ExternalOutput").ap()
    with tile.TileContext(nc) as tc:
        _emit(tc, xT, wqk, wv, wo0, wo1, out)
    nc.compile()
    _NC_CACHE["nc"] = nc
    return nc


def make_in_maps(x, qkv_w, out_w):
    """Host-side shard + transpose + cast to fp16. Core c: batch c//4,
    heads 4*(c%4) .. 4*(c%4)+4 as two pairs."""
    f16 = np.float16
    xTb = [np.ascontiguousarray(x[b].T).astype(f16) for b in range(B)]
    maps = []
    for c in range(NCORES):
        b, hg = c // 4, c % 4
        heads = [4 * hg + j for j in range(4)]
        wq, wk, wv_ = [], [], []
        for h in heads:
            rows = qkv_w[h * 192:(h + 1) * 192]
            wq.append(rows[0:64])
            wk.append(rows[64:128])
            wv_.append(rows[128:192])
        # pair p = heads (2p, 2p+1); col layout [q_p0|q_p1|k_p0|k_p1]
        wqk_c = np.concatenate(wq + wk, 0).T  # [D, 512]
        wv_c = np.concatenate(wv_, 0).T       # [D, 256]
        f0 = hg * 256
        maps.append({
            "xT": xTb[b],
            "wqk": np.ascontiguousarray(wqk_c).astype(f16),
            "wv": np.ascontiguousarray(wv_c).astype(f16),
            "wo0": np.ascontiguousarray(out_w[:, f0:f0 + 128].T).astype(f16),
            "wo1": np.ascontiguousarray(out_w[:, f0 + 128:f0 + 256].T).astype(f16),
        })
    return maps


def kernel(x, qkv_w, out_w, out_b, _run_kwargs=None):
    x = np.asarray(x, dtype=np.float32)
    qkv_w = np.asarray(qkv_w, dtype=np.float32)
    out_w = np.asarray(out_w, dtype=np.float32)
    out_b = np.asarray(out_b, dtype=np.float32)

    nc = build_nc()
    in_maps = make_in_maps(x, qkv_w, out_w)
    res = run_bass_kernel_spmd(
        nc, in_maps, list(range(NCORES)), **(_run_kwargs or {})
    )
    total = np.zeros((B, S, D), np.float32)
    for c in range(NCORES):
        total[c // 4] += np.asarray(res.results[c]["out"], dtype=np.float32)
    total += out_b[None, None, :]
    if _run_kwargs:
        kernel.last_result = res
    return total
